# revision 8
# baseline (speedup 1.0000x reference)
"""Erosion v2: row-segment layout, no cross-partition traffic.

Partition p holds rows 8p-2 .. 8p+9 of one image as 12 free-dim
segments (order: halo -2,-1, main 0..7, halo +8,+9), each 1028 cols
(2-col pads). The 4-row halo is re-read from DRAM via strided DMAs
(partition-aligned, fast) instead of SBUF->SBUF partition-shifted
copies (which measure ~50 GB/s and dominated v1).

Vertical pass = 3 shifted tensor_tensor(min) along the segment axis,
horizontal pass = 3 shifted TTs within segments; both pure free-dim.
Column chunks of 256 keep intermediates small; x stays resident
full-width per image.
"""

import numpy as np

import concourse.bacc as bacc
import concourse.mybir as mybir
import concourse.tile as tile
from concourse.bass_utils import run_bass_kernel_spmd

B, H, W = 32, 1024, 1024
N_CORES = 8
PER_CORE = B // N_CORES     # 4 images per core
PX = 2
PAD_VAL = 1e4
F32 = mybir.dt.float32
MIN = mybir.AluOpType.min

KR = 8                      # output rows per partition (128*8 = 1024)
SEGS = KR + 2 * PX          # 12 segments per partition
WP = W + 2 * PX             # 1028 padded width
CW = 256                    # output cols per chunk
CWH = CW + 2 * PX           # 260
N_CC = W // CW              # 4

_CACHE = {}


def build_nc(repeat: int = 1):
    nc = bacc.Bacc("TRN2", debug=False, num_devices=N_CORES)
    x = nc.dram_tensor("mask", [PER_CORE, H, W], F32, kind="ExternalInput").ap()
    y = nc.dram_tensor("out", [PER_CORE, H, W], F32, kind="ExternalOutput").ap()

    with tile.TileContext(nc) as tc:
        with (
            tc.tile_pool(name="const", bufs=1) as cpool,
            tc.tile_pool(name="xp", bufs=2) as xpool,
            tc.tile_pool(name="t1", bufs=2) as t1p,
            tc.tile_pool(name="t2", bufs=2) as t2p,
            tc.tile_pool(name="vp", bufs=2) as vpool,
            tc.tile_pool(name="op", bufs=2) as opool,
        ):
            # 1e4 source for pad fills at partition 127 (memset can't start
            # there; DMA is exempt from start-partition rules)
            cpad = cpool.tile([128, 2 * WP], F32)
            nc.vector.memset(cpad[:, :], PAD_VAL)

            for rep in range(repeat):
                for img in range(PER_CORE):
                    xt = xpool.tile([128, SEGS * WP], F32, tag="x")
                    x3 = xt[:, :].rearrange("p (s c) -> p s c", s=SEGS)

                    # column pads (all segments)
                    nc.vector.memset(x3[:, :, 0:PX], PAD_VAL)
                    nc.vector.memset(x3[:, :, W + PX : WP], PAD_VAL)
                    # row pads: partition 0 segs 0,1 (rows -2,-1)
                    nc.vector.memset(x3[0:1, 0:PX, 2 : W + PX], PAD_VAL)
                    # partition 127 segs 10,11 (rows 1024,1025)
                    nc.sync.dma_start(
                        out=x3[127:128, KR + PX : SEGS, PX : W + PX],
                        in_=cpad[0:1, 0 : 2 * W],
                    )

                    # main rows: partition p segs 2..9 <- rows 8p..8p+7
                    nc.sync.dma_start(
                        out=x3[:, PX : PX + KR, PX : W + PX],
                        in_=x[img].rearrange("(p s) c -> p s c", s=KR),
                    )
                    # halo segs via strided row sampling
                    # seg0: row 8p-2 (p>=1): rows 6,14,..,1014
                    nc.sync.dma_start(
                        out=x3[1:128, 0:1, PX : W + PX],
                        in_=x[img, KR - PX : H - PX : KR, :].unsqueeze(1),
                    )
                    # seg1: row 8p-1 (p>=1): rows 7,15,..,1015
                    nc.sync.dma_start(
                        out=x3[1:128, 1:2, PX : W + PX],
                        in_=x[img, KR - 1 : H - 1 : KR, :].unsqueeze(1),
                    )
                    # seg10: row 8p+8 (p<=126): rows 8,16,..,1016
                    nc.sync.dma_start(
                        out=x3[0:127, KR + PX : KR + PX + 1, PX : W + PX],
                        in_=x[img, KR:H:KR, :].unsqueeze(1),
                    )
                    # seg11: row 8p+9 (p<=126): rows 9,17,..,1017
                    nc.sync.dma_start(
                        out=x3[0:127, KR + PX + 1 : SEGS, PX : W + PX],
                        in_=x[img, KR + 1 : H : KR, :].unsqueeze(1),
                    )

                    for cc in range(N_CC):
                        c0 = cc * CW    # resident col index of first halo col
                        xs = x3[:, :, c0 : c0 + CWH]

                        w2 = t1p.tile([128, (SEGS - 1) * CWH], F32, tag="t1")
                        w2_3 = w2[:, :].rearrange("p (s c) -> p s c", s=SEGS - 1)
                        nc.vector.tensor_tensor(
                            out=w2_3[:, :, :],
                            in0=xs[:, 0 : SEGS - 1, :],
                            in1=xs[:, 1:SEGS, :],
                            op=MIN,
                        )
                        w4 = t2p.tile([128, (SEGS - 3) * CWH], F32, tag="t2")
                        w4_3 = w4[:, :].rearrange("p (s c) -> p s c", s=SEGS - 3)
                        nc.vector.tensor_tensor(
                            out=w4_3[:, :, :],
                            in0=w2_3[:, 0 : SEGS - 3, :],
                            in1=w2_3[:, 2 : SEGS - 1, :],
                            op=MIN,
                        )
                        v = vpool.tile([128, KR * CWH], F32, tag="v")
                        v3 = v[:, :].rearrange("p (s c) -> p s c", s=KR)
                        nc.vector.tensor_tensor(
                            out=v3[:, :, :],
                            in0=w4_3[:, 0:KR, :],
                            in1=xs[:, 2 * PX : SEGS, :],
                            op=MIN,
                        )

                        a = t1p.tile([128, KR * (CWH - 1)], F32, tag="t1")
                        a3 = a[:, :].rearrange("p (s c) -> p s c", s=KR)
                        nc.vector.tensor_tensor(
                            out=a3[:, :, :],
                            in0=v3[:, :, 0 : CWH - 1],
                            in1=v3[:, :, 1:CWH],
                            op=MIN,
                        )
                        bb = t2p.tile([128, KR * (CWH - 3)], F32, tag="t2")
                        b3 = bb[:, :].rearrange("p (s c) -> p s c", s=KR)
                        nc.vector.tensor_tensor(
                            out=b3[:, :, :],
                            in0=a3[:, :, 0 : CWH - 3],
                            in1=a3[:, :, 2 : CWH - 1],
                            op=MIN,
                        )
                        o = opool.tile([128, KR * CW], F32, tag="o")
                        o3 = o[:, :].rearrange("p (s c) -> p s c", s=KR)
                        nc.vector.tensor_tensor(
                            out=o3[:, :, :],
                            in0=b3[:, :, 0:CW],
                            in1=v3[:, :, 2 * PX : CWH],
                            op=MIN,
                        )

                        nc.sync.dma_start(
                            out=y[img, :, cc * CW : (cc + 1) * CW].rearrange(
                                "(p s) c -> p s c", s=KR
                            ),
                            in_=o3[:, :, :],
                        )

    nc.compile()
    return nc


def run(mask: np.ndarray, trace: bool = False):
    assert mask.shape == (B, 1, H, W), mask.shape
    in_dtype = mask.dtype
    mask4 = np.ascontiguousarray(
        mask.reshape(B, H, W).astype(np.float32, copy=False)
    )
    if "nc" not in _CACHE:
        _CACHE["nc"] = build_nc(1)
    nc = _CACHE["nc"]
    in_maps = [
        {"mask": mask4[i * PER_CORE : (i + 1) * PER_CORE]} for i in range(N_CORES)
    ]
    res = run_bass_kernel_spmd(nc, in_maps, list(range(N_CORES)), trace=trace)
    out = np.concatenate([res.results[i]["out"] for i in range(N_CORES)], axis=0)
    return out.reshape(B, 1, H, W).astype(in_dtype, copy=False), res


def kernel(mask: np.ndarray) -> np.ndarray:
    return run(mask)[0]
